# revision 30
# baseline (speedup 1.0000x reference)
"""Trainium2 Bass kernel for causal multi-head attention block.

Reference computation (fp32):
    qkv = x @ w_qkv;  q,k,v = split(qkv)
    attn = softmax(causal_mask(q k^T / sqrt(64)))
    out  = (attn @ v reassembled) @ w_out

Sharding over 8 NeuronCores: core c handles batch b = c//4 and heads
4*(c%4) .. 4*(c%4)+3 (4 of 16 heads).  Each core computes the rank-256
partial product of the output projection restricted to its heads'
channels; the host sums the 4 partials per batch.

All-fp16 dataflow (inputs converted on host; fp32 PSUM accumulation),
block-interleaved schedule: for each 512-row T block, project q/k/v,
then run attention for that query block (whose keys are now all
available), with the output projection lagged one block.  This keeps
the PE dense across phase boundaries (HAM clock-throttle stays
released) and hides the softmax exp (ACT-bound inner loop, ~1.0us per
128x1024 tile) and the normalization chains under projection matmuls.
Softmax denominators ride the O accumulation as a fused 65th
stationary column; mid-kernel 1/den goes through a DRAM scatter
round-trip + DVE reciprocal (off the PE's in-order instruction
stream), while the final chain - which is on the critical path - uses
a K=1 ones-matmul partition broadcast + single-op fast-approx DVE
reciprocal.  Measured ~169-175us per core on TRN2 (scale-relative max
err ~5e-4 vs the fp32 reference; baseline was ~194-199us).
"""

import sys

for _p in ("/opt/trn_rl_repo", "/root/.axon_site/_ro/trn_rl_repo"):
    if _p not in sys.path:
        sys.path.append(_p)

import numpy as np

import concourse.bass as bass
import concourse.mybir as mybir
import concourse.tile as tile
from concourse import bacc, bass_utils

P = 128
B, T, C = 2, 2048, 1024
HPC = 4            # heads per core
DH = 64            # head dim
KT = C // P        # 8 contraction tiles over d_model
NQB = T // 512     # 4 query blocks of 512
NKT = T // P       # 16 key tiles of 128
F32 = mybir.dt.float32
F16 = mybir.dt.float16
EXP = mybir.ActivationFunctionType.Exp
LOG = mybir.ActivationFunctionType.Ln
SCALE = 1.0 / 8.0  # 1/sqrt(DH)


def _body(tc, nc, xT, wq, wk, wv, wo, tri, vones, out):
    with tc.tile_pool(name="const", bufs=1) as cpool:
        wq_sb = cpool.tile([P, KT, 2 * P], F16, name="wq_sb")
        wk_sb = cpool.tile([P, KT, 2 * P], F16, name="wk_sb")
        wv_sb = cpool.tile([P, KT, 2 * P], F16, name="wv_sb")
        wo_sb = cpool.tile([P, 2, C], F16, name="wo_sb")
        tri_sb = cpool.tile([P, P], F16, name="tri_sb")
        xTv = xT.rearrange("(kt p) t -> p kt t", p=P)

        # persistent stores
        qT = [cpool.tile([P, T], F16, name=f"qT{pr}") for pr in range(2)]
        kT = [cpool.tile([P, T], F16, name=f"kT{pr}") for pr in range(2)]
        vS = cpool.tile([P, NKT, HPC, DH + 1], F16, name="vS")
        oT = [cpool.tile([P, T], F16, name=f"oT{pr}") for pr in range(2)]
        xhs = [cpool.tile([P, KT, 1024], F16, name=f"xh{i}") for i in range(2)]

        # ---- startup DMA: first q chain's inputs first, then the rest.
        # Weight tensors are pre-transposed on host so every transfer moves
        # >=2KB contiguous per partition line (DMA efficiency knee).
        nc.sync.dma_start(wq_sb[:, 0:2], wq[:, 0:2])
        nc.sync.dma_start(xhs[0][:, 0:2, :], xTv[:, 0:2, 0:1024])
        nc.sync.dma_start(wq_sb[:, 2:8], wq[:, 2:8])
        nc.sync.dma_start(xhs[0][:, 2:8, :], xTv[:, 2:8, 0:1024])
        nc.sync.dma_start(wk_sb, wk)
        nc.sync.dma_start(tri_sb, tri)
        nc.sync.dma_start(vS[:, :, :, DH : DH + 1], vones)
        nc.sync.dma_start(wv_sb, wv)
        nc.sync.dma_start(xhs[1], xTv[:, :, 1024:2048])
        nc.gpsimd.dma_start(wo_sb, wo)

        # preload the exp ACT table set during the startup DMA window
        warm = cpool.tile([1, 2], F32, name="warm")
        nc.vector.memset(warm, 1.0)
        nc.scalar.activation(warm, warm, EXP, scale=1.0)
        ones1 = cpool.tile([1, DH], F16, name="ones1")
        nc.vector.memset(ones1, 1.0)

        with (
            tc.tile_pool(name="ptp", bufs=8) as ptp,
            tc.tile_pool(name="nrm", bufs=2) as nrm,
            tc.tile_pool(name="dsc", bufs=2, space="DRAM") as dsc,
            tc.tile_pool(name="osb", bufs=4) as osb,
        ):
            # ---------- helpers shared by both PSUM phases ----------
            def geom(qb, j):
                r = j - 4 * qb
                width = 512 - r * P if r >= 0 else 512
                col0 = r * P if r >= 0 else 0
                return r, width, col0

            def s_exp_mask(qb, pr, j, pts, spool, tag):
                r, width, col0 = geom(qb, j)
                qoff = qb * 512 + col0
                sp_ = spool.tile([P, 1024], F32, name=tag, tag=tag)
                for h in range(2):
                    nc.tensor.matmul(
                        sp_[:, h * 512 : h * 512 + width],
                        kT[pr][h * DH : (h + 1) * DH, j * P : (j + 1) * P],
                        qT[pr][h * DH : (h + 1) * DH, qoff : qoff + width],
                        start=True,
                        stop=True,
                    )
                pt = ptp.tile([P, 1024], F16, name="pt")
                s3 = sp_.rearrange("p (h w) -> p h w", h=2)[:, :, 0:width]
                p3 = pt.rearrange("p (h w) -> p h w", h=2)[:, :, 0:width]
                nc.scalar.activation(p3, s3, EXP, scale=SCALE)
                if r >= 0:
                    for h in range(2):
                        nc.gpsimd.tensor_mul(
                            pt[:, h * 512 : h * 512 + P],
                            pt[:, h * 512 : h * 512 + P],
                            tri_sb,
                        )
                pts[j] = pt

            def o_mm(qb, pr, j, nk, op, pts):
                _, width, col0 = geom(qb, j)
                pt = pts.pop(j)
                for h in range(2):
                    nc.tensor.matmul(
                        op[0 : DH + 1, h * 512 + col0 : (h + 1) * 512],
                        vS[:, j, pr * 2 + h, :],
                        pt[:, h * 512 : h * 512 + width],
                        start=(j == 0),
                        stop=(j == nk - 1),
                        skip_group_check=True,
                    )

            def norm_tail(qb, pr, op, bc):
                """normalized oT from the finished accumulator + bc recips."""
                qs = slice(qb * 512, (qb + 1) * 512)
                oTu = nrm.tile([DH, 2, 512], F16, name="oTu", tag="oTu")
                nc.vector.tensor_copy(
                    oTu, op.rearrange("p (h w) -> p h w", h=2)[0:DH]
                )
                nc.vector.tensor_mul(oT[pr][0:DH, qs], oTu[0:DH, 0, :], bc[:, 0:512])
                o1 = nrm.tile([DH, 512], F16, name="o1", tag="o1")
                nc.vector.tensor_mul(o1, oTu[0:DH, 1, :], bc[:, 512:1024])
                nc.sync.dma_start(oT[pr][DH : 2 * DH, qs], o1)
            def emit_qkv(tb, psum):
                """q/k/v projections for T block tb (both head pairs)."""
                xt = xhs[tb // 2]
                c0 = (tb % 2) * 512
                ts = slice(tb * 512, (tb + 1) * 512)
                for pr, (w_sb, dst) in (
                    (0, (wq_sb, qT)),
                    (0, (wk_sb, kT)),
                    (1, (wq_sb, qT)),
                    (1, (wk_sb, kT)),
                ):
                    if True:
                        ps = psum.tile([P, 512], F32, name="qk", tag="qk")
                        for kt in range(KT):
                            nc.tensor.matmul(
                                ps,
                                w_sb[:, kt, pr * P : (pr + 1) * P],
                                xt[:, kt, c0 : c0 + 512],
                                start=(kt == 0),
                                stop=(kt == KT - 1),
                            )
                        nc.vector.tensor_copy(dst[pr][:, ts], ps)
                for sub in range(4):
                    tb1 = tb * 4 + sub
                    vp = psum.tile([P, 512], F32, name="qk", tag="qk")
                    for kt in range(KT):
                        nc.tensor.matmul(
                            vp[:, 0 : 2 * P],
                            xt[:, kt, c0 + sub * P : c0 + (sub + 1) * P],
                            wv_sb[:, kt, :],
                            start=(kt == 0),
                            stop=(kt == KT - 1),
                        )
                    nc.vector.tensor_copy(
                        vS[:, tb1, :, 0:DH],
                        vp[:, 0 : 2 * P].rearrange("p (h d) -> p h d", d=DH),
                    )

            def emit_proj(tb, psum, tag="qk"):
                """output projection for T block tb (4 row tiles of 128)."""
                for tb1 in range(tb * 4, tb * 4 + 4):
                    ot = osb.tile([P, 1024], F16, name="ot")
                    for cb in range(2):
                        pp = psum.tile([P, 512], F32, name=tag, tag=tag)
                        for pr in range(2):
                            nc.tensor.matmul(
                                pp,
                                oT[pr][:, tb1 * P : (tb1 + 1) * P],
                                wo_sb[:, pr, cb * 512 : (cb + 1) * 512],
                                start=(pr == 0),
                                stop=(pr == 1),
                            )
                        nc.vector.tensor_copy(
                            ot[:, cb * 512 : (cb + 1) * 512], pp
                        )
                    nc.sync.dma_start(out[tb1 * P : (tb1 + 1) * P, :], ot)

            def den_recip_dma(op, use_act=True):
                """1/den via DRAM-round-trip scatter + DVE reciprocal."""
                dTu = nrm.tile([1, 1024], F16, name="dTu", tag="dTu")
                if use_act:
                    nc.scalar.copy(dTu, op[DH : DH + 1, :])
                else:
                    nc.vector.tensor_copy(dTu, op[DH : DH + 1, :])
                dd = dsc.tile([1024], F16, name="dd", tag="dd")
                nc.sync.dma_start(dd[None], dTu)
                rsh = nrm.tile([P, 8], F16, name="rsh", tag="rsh")
                nc.sync.dma_start(rsh, dd.rearrange("(p c) -> p c", p=P))
                rr = nrm.tile([P, 8], F16, name="rr", tag="rr")
                with nc.allow_low_precision(reason="fp16 softmax denom"):
                    nc.vector.reciprocal(rr, rsh)
                dd2 = dsc.tile([1024], F16, name="dd2", tag="dd2")
                nc.sync.dma_start(dd2.rearrange("(p c) -> p c", p=P), rr)
                bc = nrm.tile([DH, 1024], F16, name="bch", tag="bch")
                nc.sync.dma_start(
                    bc[:, 0:512], dd2[None, 0:512].to_broadcast([DH, 512])
                )
                nc.sync.dma_start(
                    bc[:, 512:1024], dd2[None, 512:1024].to_broadcast([DH, 512])
                )
                return bc

            def den_recip_fast(op, psum, tag):
                """1/den via K=1 PE broadcast + fast-approx DVE reciprocal."""
                dTu = nrm.tile([1, 1024], F16, name="dTu", tag="dTu")
                nc.scalar.copy(dTu, op[DH : DH + 1, :])
                bc = nrm.tile([DH, 1024], F32, name="bc", tag="bc")
                for half in range(2):
                    hs = slice(half * 512, (half + 1) * 512)
                    bcp = psum.tile([P, 512], F32, name=tag, tag=tag)
                    nc.tensor.matmul(
                        bcp[0:DH, :], ones1, dTu[:, hs], start=True, stop=True
                    )
                    nc.vector.reciprocal_approx_fast(bc[:, hs], bcp[0:DH, :])
                return bc

            # ---------- main schedule ----------
            with (
                tc.tile_pool(name="qkp", bufs=2, space="PSUM") as qkp,
                tc.tile_pool(name="sps", bufs=2, space="PSUM") as sps,
                tc.tile_pool(name="ops", bufs=1, space="PSUM") as ops,
            ):
                # PE warm-up: dummy matmuls on a zero tile during the startup
                # DMA window, so HAM releases the throttle before real work.
                wdum = cpool.tile([P, DH], F16, name="wdum")
                nc.vector.memset(wdum, 0.0)
                dum = qkp.tile([P, 512], F32, name="qk", tag="qk")
                for i in range(80):
                    nc.tensor.matmul(
                        dum[0:DH, 0:DH], wdum, wdum, start=(i == 0), stop=(i == 79)
                    )

                for tb in range(NQB):
                    emit_qkv(tb, qkp)
                    for pr in range(2):
                        op = ops.tile([DH + 1, 1024], F32, name="op", tag="op")
                        nk = 4 * tb + 4
                        pts = {}
                        for j in range(nk):
                            s_exp_mask(tb, pr, j, pts, sps, "sp")
                            if j > 1:
                                o_mm(tb, pr, j - 2, nk, op, pts)
                        for jj in range(max(0, nk - 2), nk):
                            o_mm(tb, pr, jj, nk, op, pts)
                        if tb == NQB - 1 and pr == 1:
                            # the final chain is on the critical path: use the
                            # PE-broadcast + fast-approx reciprocal (no DRAM
                            # hops); mid-kernel chains use the DMA round trip
                            # so the PE stream never waits on them.
                            bc = den_recip_fast(op, qkp, "qk")
                        else:
                            bc = den_recip_dma(op, use_act=(pr == 1))
                        norm_tail(tb, pr, op, bc)
                    if tb >= 1:
                        emit_proj(tb - 1, qkp)
                emit_proj(NQB - 1, qkp)


def build_bass():
    nc = bacc.Bacc("TRN2", target_bir_lowering=False, debug=False, num_devices=8)
    xT = nc.dram_tensor("xT", [C, T], F16, kind="ExternalInput").ap()
    wq = nc.dram_tensor("wq", [P, KT, 2 * P], F16, kind="ExternalInput").ap()
    wk = nc.dram_tensor("wk", [P, KT, 2 * P], F16, kind="ExternalInput").ap()
    wv = nc.dram_tensor("wv", [P, KT, 2 * P], F16, kind="ExternalInput").ap()
    wo = nc.dram_tensor("wo", [P, 2, C], F16, kind="ExternalInput").ap()
    tri = nc.dram_tensor("tri", [P, P], F16, kind="ExternalInput").ap()
    vones = nc.dram_tensor(
        "vones", [P, NKT, HPC, 1], F16, kind="ExternalInput"
    ).ap()
    out = nc.dram_tensor("out", [T, C], F16, kind="ExternalOutput").ap()
    with tile.TileContext(nc) as tc:
        _body(tc, nc, xT, wq, wk, wv, wo, tri, vones, out)
    nc.compile()
    return nc


def make_in_maps(x, w_qkv, w_out):
    """Host-side sharding: returns the 8 per-core input dicts."""
    x = np.asarray(x, dtype=np.float32)
    w_qkv = np.asarray(w_qkv, dtype=np.float16)
    w_out = np.asarray(w_out, dtype=np.float16)
    def _wt(w):
        # [KT*P, n] -> [P, KT, n] so each partition row is DRAM-contiguous
        return np.ascontiguousarray(w.reshape(KT, P, 2 * P).transpose(1, 0, 2))

    kk = np.arange(P)
    tri = (kk[None, :] >= kk[:, None]).astype(np.float16)  # [k, q]: q >= k
    xTb = [np.ascontiguousarray(x[b].T.astype(np.float16)) for b in range(B)]
    in_maps = []
    for c in range(8):
        b = c // 4
        g = c % 4
        h0 = HPC * g * DH  # 256*g
        in_maps.append(
            {
                "xT": xTb[b],
                "wq": _wt(w_qkv[:, h0 : h0 + 2 * P]),
                "wk": _wt(w_qkv[:, C + h0 : C + h0 + 2 * P]),
                "wv": _wt(w_qkv[:, 2 * C + h0 : 2 * C + h0 + 2 * P]),
                "wo": np.ascontiguousarray(
                    w_out[h0 : h0 + 2 * P, :].reshape(2, P, C).transpose(1, 0, 2)
                ),
                "tri": np.ascontiguousarray(tri),
                "vones": np.ones((P, NKT, HPC, 1), dtype=np.float16),
            }
        )
    return in_maps


_NC_CACHE = None
LAST_RESULTS = None  # BassKernelResults of the most recent run (for profiling)
TRACE = False


def kernel(x, w_qkv, w_out):
    global _NC_CACHE, LAST_RESULTS
    if _NC_CACHE is None:
        _NC_CACHE = build_bass()
    nc = _NC_CACHE
    in_maps = make_in_maps(x, w_qkv, w_out)
    res = bass_utils.run_bass_kernel_spmd(
        nc, in_maps, core_ids=list(range(8)), trace=TRACE
    )
    LAST_RESULTS = res
    out = np.zeros((B, T, C), dtype=np.float32)
    for c in range(8):
        out[c // 4] += res.results[c]["out"].astype(np.float32)
    return out


if __name__ == "__main__":
    # smoke test with random data
    rng = np.random.default_rng(0)
    x = rng.standard_normal((B, T, C), dtype=np.float32)
    w_qkv = rng.standard_normal((C, 3 * C), dtype=np.float32) / np.sqrt(C)
    w_out = rng.standard_normal((C, C), dtype=np.float32) / np.sqrt(C)
    o = kernel(x, w_qkv, w_out)
    print(o.shape, o.dtype)


# revision 31
# speedup vs baseline: 1.2086x; 1.2086x over previous
"""Trainium2 Bass kernel for causal multi-head attention block.

Reference computation (fp32):
    qkv = x @ w_qkv;  q,k,v = split(qkv)
    attn = softmax(causal_mask(q k^T / sqrt(64)))
    out  = (attn @ v reassembled) @ w_out

Sharding over 8 NeuronCores: core c handles batch b = c//4 and heads
4*(c%4) .. 4*(c%4)+3 (4 of 16 heads).  Each core computes the rank-256
partial product of the output projection restricted to its heads'
channels; the host sums the 4 partials per batch.

All-fp16 dataflow (inputs converted on host; fp32 PSUM accumulation),
block-interleaved schedule: for each 512-row T block, project q/k/v,
then run attention for that query block (whose keys are now all
available), with the output projection lagged one block.  This keeps
the PE dense across phase boundaries (HAM clock-throttle stays
released) and hides the softmax exp (ACT-bound inner loop, ~1.0us per
128x1024 tile) and the normalization chains under projection matmuls.
Softmax denominators ride the O accumulation as a fused 65th
stationary column; mid-kernel 1/den goes through a DRAM scatter
round-trip + DVE reciprocal (off the PE's in-order instruction
stream), while the final chain - which is on the critical path - uses
a K=1 ones-matmul partition broadcast + single-op fast-approx DVE
reciprocal.  Measured ~169-175us per core on TRN2 (scale-relative max
err ~5e-4 vs the fp32 reference; baseline was ~194-199us).
"""

import sys

for _p in ("/opt/trn_rl_repo", "/root/.axon_site/_ro/trn_rl_repo"):
    if _p not in sys.path:
        sys.path.append(_p)

import numpy as np

import concourse.bass as bass
import concourse.mybir as mybir
import concourse.tile as tile
from concourse import bacc, bass_utils

P = 128
B, T, C = 2, 2048, 1024
HPC = 4            # heads per core
DH = 64            # head dim
KT = C // P        # 8 contraction tiles over d_model
NQB = T // 512     # 4 query blocks of 512
NKT = T // P       # 16 key tiles of 128
F32 = mybir.dt.float32
F16 = mybir.dt.float16
EXP = mybir.ActivationFunctionType.Exp
LOG = mybir.ActivationFunctionType.Ln
SCALE = 1.0 / 8.0  # 1/sqrt(DH)


def _body(tc, nc, xT, wq, wk, wv, wo, tri, vones, out):
    with tc.tile_pool(name="const", bufs=1) as cpool:
        wq_sb = cpool.tile([P, KT, 2 * P], F16, name="wq_sb")
        wk_sb = cpool.tile([P, KT, 2 * P], F16, name="wk_sb")
        wv_sb = cpool.tile([P, KT, 2 * P], F16, name="wv_sb")
        wo_sb = cpool.tile([P, 2, C], F16, name="wo_sb")
        tri_sb = cpool.tile([P, P], F16, name="tri_sb")
        xTv = xT.rearrange("(kt p) t -> p kt t", p=P)

        # persistent stores
        qT = [cpool.tile([P, T], F16, name=f"qT{pr}") for pr in range(2)]
        kT = [cpool.tile([P, T], F16, name=f"kT{pr}") for pr in range(2)]
        vS = cpool.tile([P, NKT, HPC, DH + 1], F16, name="vS")
        oT = [cpool.tile([P, T], F16, name=f"oT{pr}") for pr in range(2)]
        xhs = [cpool.tile([P, KT, 1024], F16, name=f"xh{i}") for i in range(2)]

        # ---- startup DMA: first q chain's inputs first, then the rest.
        # Weight tensors are pre-transposed on host so every transfer moves
        # >=2KB contiguous per partition line (DMA efficiency knee).
        nc.sync.dma_start(wq_sb[:, 0:2], wq[:, 0:2])
        nc.sync.dma_start(xhs[0][:, 0:2, :], xTv[:, 0:2, 0:1024])
        nc.sync.dma_start(wq_sb[:, 2:8], wq[:, 2:8])
        nc.sync.dma_start(xhs[0][:, 2:8, :], xTv[:, 2:8, 0:1024])
        nc.sync.dma_start(wk_sb, wk)
        nc.sync.dma_start(tri_sb, tri)
        nc.sync.dma_start(vS[:, :, :, DH : DH + 1], vones)
        nc.sync.dma_start(wv_sb, wv)
        nc.sync.dma_start(xhs[1], xTv[:, :, 1024:2048])
        nc.gpsimd.dma_start(wo_sb, wo)

        # preload the exp ACT table set during the startup DMA window
        warm = cpool.tile([1, 2], F32, name="warm")
        nc.vector.memset(warm, 1.0)
        nc.scalar.activation(warm, warm, EXP, scale=1.0)
        ones1 = cpool.tile([1, DH], F16, name="ones1")
        nc.vector.memset(ones1, 1.0)

        with (
            tc.tile_pool(name="ptp", bufs=8) as ptp,
            tc.tile_pool(name="nrm", bufs=2) as nrm,
            tc.tile_pool(name="dsc", bufs=2, space="DRAM") as dsc,
            tc.tile_pool(name="osb", bufs=4) as osb,
        ):
            # ---------- helpers shared by both PSUM phases ----------
            def geom(qb, j):
                r = j - 4 * qb
                width = 512 - r * P if r >= 0 else 512
                col0 = r * P if r >= 0 else 0
                return r, width, col0

            def s_exp_mask(qb, pr, j, pts, spool, tag):
                r, width, col0 = geom(qb, j)
                qoff = qb * 512 + col0
                sp_ = spool.tile([P, 1024], F32, name=tag, tag=tag)
                for h in range(2):
                    nc.tensor.matmul(
                        sp_[:, h * 512 : h * 512 + width],
                        kT[pr][h * DH : (h + 1) * DH, j * P : (j + 1) * P],
                        qT[pr][h * DH : (h + 1) * DH, qoff : qoff + width],
                        start=True,
                        stop=True,
                    )
                pt = ptp.tile([P, 1024], F16, name="pt")
                s3 = sp_.rearrange("p (h w) -> p h w", h=2)[:, :, 0:width]
                p3 = pt.rearrange("p (h w) -> p h w", h=2)[:, :, 0:width]
                nc.scalar.activation(p3, s3, EXP, scale=SCALE)
                if r >= 0:
                    for h in range(2):
                        nc.vector.tensor_mul(
                            pt[:, h * 512 : h * 512 + P],
                            pt[:, h * 512 : h * 512 + P],
                            tri_sb,
                        )
                pts[j] = pt

            def o_mm(qb, pr, j, nk, op, pts):
                _, width, col0 = geom(qb, j)
                pt = pts.pop(j)
                for h in range(2):
                    nc.tensor.matmul(
                        op[0 : DH + 1, h * 512 + col0 : (h + 1) * 512],
                        vS[:, j, pr * 2 + h, :],
                        pt[:, h * 512 : h * 512 + width],
                        start=(j == 0),
                        stop=(j == nk - 1),
                        skip_group_check=True,
                    )

            def norm_tail(qb, pr, op, bc):
                """normalized oT from the finished accumulator + bc recips."""
                qs = slice(qb * 512, (qb + 1) * 512)
                oTu = nrm.tile([DH, 2, 512], F16, name="oTu", tag="oTu")
                nc.vector.tensor_copy(
                    oTu, op.rearrange("p (h w) -> p h w", h=2)[0:DH]
                )
                nc.vector.tensor_mul(oT[pr][0:DH, qs], oTu[0:DH, 0, :], bc[:, 0:512])
                o1 = nrm.tile([DH, 512], F16, name="o1", tag="o1")
                nc.vector.tensor_mul(o1, oTu[0:DH, 1, :], bc[:, 512:1024])
                nc.sync.dma_start(oT[pr][DH : 2 * DH, qs], o1)
            def emit_qkv(tb, psum):
                """q/k/v projections for T block tb (both head pairs)."""
                xt = xhs[tb // 2]
                c0 = (tb % 2) * 512
                ts = slice(tb * 512, (tb + 1) * 512)
                for pr, (w_sb, dst) in (
                    (0, (wq_sb, qT)),
                    (0, (wk_sb, kT)),
                    (1, (wq_sb, qT)),
                    (1, (wk_sb, kT)),
                ):
                    if True:
                        ps = psum.tile([P, 512], F32, name="qk", tag="qk")
                        for kt in range(KT):
                            nc.tensor.matmul(
                                ps,
                                w_sb[:, kt, pr * P : (pr + 1) * P],
                                xt[:, kt, c0 : c0 + 512],
                                start=(kt == 0),
                                stop=(kt == KT - 1),
                            )
                        nc.vector.tensor_copy(dst[pr][:, ts], ps)
                for sub in range(4):
                    tb1 = tb * 4 + sub
                    vp = psum.tile([P, 512], F32, name="qk", tag="qk")
                    for kt in range(KT):
                        nc.tensor.matmul(
                            vp[:, 0 : 2 * P],
                            xt[:, kt, c0 + sub * P : c0 + (sub + 1) * P],
                            wv_sb[:, kt, :],
                            start=(kt == 0),
                            stop=(kt == KT - 1),
                        )
                    nc.vector.tensor_copy(
                        vS[:, tb1, :, 0:DH],
                        vp[:, 0 : 2 * P].rearrange("p (h d) -> p h d", d=DH),
                    )

            def emit_proj(tb, psum, tag="qk"):
                """output projection for T block tb (4 row tiles of 128)."""
                for tb1 in range(tb * 4, tb * 4 + 4):
                    ot = osb.tile([P, 1024], F16, name="ot")
                    for cb in range(2):
                        pp = psum.tile([P, 512], F32, name=tag, tag=tag)
                        for pr in range(2):
                            nc.tensor.matmul(
                                pp,
                                oT[pr][:, tb1 * P : (tb1 + 1) * P],
                                wo_sb[:, pr, cb * 512 : (cb + 1) * 512],
                                start=(pr == 0),
                                stop=(pr == 1),
                            )
                        nc.vector.tensor_copy(
                            ot[:, cb * 512 : (cb + 1) * 512], pp
                        )
                    nc.sync.dma_start(out[tb1 * P : (tb1 + 1) * P, :], ot)

            def den_recip_dma(op, use_act=True):
                """1/den via DRAM-round-trip scatter + DVE reciprocal."""
                dTu = nrm.tile([1, 1024], F16, name="dTu", tag="dTu")
                if use_act:
                    nc.scalar.copy(dTu, op[DH : DH + 1, :])
                else:
                    nc.vector.tensor_copy(dTu, op[DH : DH + 1, :])
                dd = dsc.tile([1024], F16, name="dd", tag="dd")
                nc.sync.dma_start(dd[None], dTu)
                rsh = nrm.tile([P, 8], F16, name="rsh", tag="rsh")
                nc.sync.dma_start(rsh, dd.rearrange("(p c) -> p c", p=P))
                rr = nrm.tile([P, 8], F16, name="rr", tag="rr")
                with nc.allow_low_precision(reason="fp16 softmax denom"):
                    nc.vector.reciprocal(rr, rsh)
                dd2 = dsc.tile([1024], F16, name="dd2", tag="dd2")
                nc.sync.dma_start(dd2.rearrange("(p c) -> p c", p=P), rr)
                bc = nrm.tile([DH, 1024], F16, name="bch", tag="bch")
                nc.sync.dma_start(
                    bc[:, 0:512], dd2[None, 0:512].to_broadcast([DH, 512])
                )
                nc.sync.dma_start(
                    bc[:, 512:1024], dd2[None, 512:1024].to_broadcast([DH, 512])
                )
                return bc

            def den_recip_fast(op, psum, tag):
                """1/den via K=1 PE broadcast + fast-approx DVE reciprocal."""
                dTu = nrm.tile([1, 1024], F16, name="dTu", tag="dTu")
                nc.scalar.copy(dTu, op[DH : DH + 1, :])
                bc = nrm.tile([DH, 1024], F32, name="bc", tag="bc")
                for half in range(2):
                    hs = slice(half * 512, (half + 1) * 512)
                    bcp = psum.tile([P, 512], F32, name=tag, tag=tag)
                    nc.tensor.matmul(
                        bcp[0:DH, :], ones1, dTu[:, hs], start=True, stop=True
                    )
                    nc.vector.reciprocal_approx_fast(bc[:, hs], bcp[0:DH, :])
                return bc

            # ---------- main schedule ----------
            with (
                tc.tile_pool(name="qkp", bufs=2, space="PSUM") as qkp,
                tc.tile_pool(name="sps", bufs=2, space="PSUM") as sps,
                tc.tile_pool(name="ops", bufs=1, space="PSUM") as ops,
            ):
                # PE warm-up: dummy matmuls on a zero tile during the startup
                # DMA window, so HAM releases the throttle before real work.
                wdum = cpool.tile([P, DH], F16, name="wdum")
                nc.vector.memset(wdum, 0.0)
                dum = qkp.tile([P, 512], F32, name="qk", tag="qk")
                for i in range(80):
                    nc.tensor.matmul(
                        dum[0:DH, 0:DH], wdum, wdum, start=(i == 0), stop=(i == 79)
                    )

                for tb in range(NQB):
                    emit_qkv(tb, qkp)
                    for pr in range(2):
                        op = ops.tile([DH + 1, 1024], F32, name="op", tag="op")
                        nk = 4 * tb + 4
                        pts = {}
                        for j in range(nk):
                            s_exp_mask(tb, pr, j, pts, sps, "sp")
                            if j > 1:
                                o_mm(tb, pr, j - 2, nk, op, pts)
                        for jj in range(max(0, nk - 2), nk):
                            o_mm(tb, pr, jj, nk, op, pts)
                        if tb == NQB - 1 and pr == 1:
                            # the final chain is on the critical path: use the
                            # PE-broadcast + fast-approx reciprocal (no DRAM
                            # hops); mid-kernel chains use the DMA round trip
                            # so the PE stream never waits on them.
                            bc = den_recip_fast(op, qkp, "qk")
                        else:
                            bc = den_recip_dma(op, use_act=(pr == 1))
                        norm_tail(tb, pr, op, bc)
                    if tb >= 1:
                        emit_proj(tb - 1, qkp)
                emit_proj(NQB - 1, qkp)


def build_bass():
    nc = bacc.Bacc("TRN2", target_bir_lowering=False, debug=False, num_devices=8)
    xT = nc.dram_tensor("xT", [C, T], F16, kind="ExternalInput").ap()
    wq = nc.dram_tensor("wq", [P, KT, 2 * P], F16, kind="ExternalInput").ap()
    wk = nc.dram_tensor("wk", [P, KT, 2 * P], F16, kind="ExternalInput").ap()
    wv = nc.dram_tensor("wv", [P, KT, 2 * P], F16, kind="ExternalInput").ap()
    wo = nc.dram_tensor("wo", [P, 2, C], F16, kind="ExternalInput").ap()
    tri = nc.dram_tensor("tri", [P, P], F16, kind="ExternalInput").ap()
    vones = nc.dram_tensor(
        "vones", [P, NKT, HPC, 1], F16, kind="ExternalInput"
    ).ap()
    out = nc.dram_tensor("out", [T, C], F16, kind="ExternalOutput").ap()
    with tile.TileContext(nc) as tc:
        _body(tc, nc, xT, wq, wk, wv, wo, tri, vones, out)
    nc.compile()
    return nc


def make_in_maps(x, w_qkv, w_out):
    """Host-side sharding: returns the 8 per-core input dicts."""
    x = np.asarray(x, dtype=np.float32)
    w_qkv = np.asarray(w_qkv, dtype=np.float16)
    w_out = np.asarray(w_out, dtype=np.float16)
    def _wt(w):
        # [KT*P, n] -> [P, KT, n] so each partition row is DRAM-contiguous
        return np.ascontiguousarray(w.reshape(KT, P, 2 * P).transpose(1, 0, 2))

    kk = np.arange(P)
    tri = (kk[None, :] >= kk[:, None]).astype(np.float16)  # [k, q]: q >= k
    xTb = [np.ascontiguousarray(x[b].T.astype(np.float16)) for b in range(B)]
    in_maps = []
    for c in range(8):
        b = c // 4
        g = c % 4
        h0 = HPC * g * DH  # 256*g
        in_maps.append(
            {
                "xT": xTb[b],
                "wq": _wt(w_qkv[:, h0 : h0 + 2 * P]),
                "wk": _wt(w_qkv[:, C + h0 : C + h0 + 2 * P]),
                "wv": _wt(w_qkv[:, 2 * C + h0 : 2 * C + h0 + 2 * P]),
                "wo": np.ascontiguousarray(
                    w_out[h0 : h0 + 2 * P, :].reshape(2, P, C).transpose(1, 0, 2)
                ),
                "tri": np.ascontiguousarray(tri),
                "vones": np.ones((P, NKT, HPC, 1), dtype=np.float16),
            }
        )
    return in_maps


_NC_CACHE = None
LAST_RESULTS = None  # BassKernelResults of the most recent run (for profiling)
TRACE = False


def kernel(x, w_qkv, w_out):
    global _NC_CACHE, LAST_RESULTS
    if _NC_CACHE is None:
        _NC_CACHE = build_bass()
    nc = _NC_CACHE
    in_maps = make_in_maps(x, w_qkv, w_out)
    res = bass_utils.run_bass_kernel_spmd(
        nc, in_maps, core_ids=list(range(8)), trace=TRACE
    )
    LAST_RESULTS = res
    out = np.zeros((B, T, C), dtype=np.float32)
    for c in range(8):
        out[c // 4] += res.results[c]["out"].astype(np.float32)
    return out


if __name__ == "__main__":
    # smoke test with random data
    rng = np.random.default_rng(0)
    x = rng.standard_normal((B, T, C), dtype=np.float32)
    w_qkv = rng.standard_normal((C, 3 * C), dtype=np.float32) / np.sqrt(C)
    w_out = rng.standard_normal((C, C), dtype=np.float32) / np.sqrt(C)
    o = kernel(x, w_qkv, w_out)
    print(o.shape, o.dtype)
